# revision 45
# baseline (speedup 1.0000x reference)
"""Paged GQA attention (prefill + decode) for 8 Trainium2 NeuronCores.

Sharding: tensor-parallel over kv-heads. Core c owns kv-head c and its 4 GQA
query heads. Block tables / context lens are baked into the program (compiled
per call), so all control flow and gather addresses are static.

v2 design (vs the 219us baseline):
  - all prefill operands fp16: halves input DMA and enables FWL weight loads
    (fp32r stationaries can't use FWL, serializing 107ns LDWEIGHTS per matmul)
  - QK matmuls causally column-sliced (25% fewer PE columns)
  - exp merged across 2 query heads per instruction (each ACTIVATE carries a
    ~352-cycle fixed overhead; merging halves the count) with causal column
    skipping via 3D strided APs
  - outputs written unnormalized (with the ones-column row sums) as fp16;
    the division happens on host: frees ~90us of VectorE work and halves
    output DMA
  - decode uses fp8e4m3 KV packed at 128-token granularity, loaded in 2 large
    DMAs at program start (instead of 64 small ones), and runs as a separate
    phase at the end so prefill PSUM pools can be released and reused
"""

import sys

if "/opt/trn_rl_repo" not in sys.path:
    sys.path.insert(0, "/opt/trn_rl_repo")

import numpy as np
import ml_dtypes

import concourse.bass as bass  # noqa: F401  (registers AP machinery)
import concourse.mybir as mybir
import concourse.tile as tile
from concourse import bacc
from concourse.bass_utils import run_bass_kernel_spmd

NUM_HEADS = 32
NUM_KV_HEADS = 8
HEAD_DIM = 128
GQA = NUM_HEADS // NUM_KV_HEADS  # 4
SCALE = 0.08838834764831845
NUM_SEQS = 4
SEQLEN = 1024
N_PREFILL = NUM_SEQS * SEQLEN  # 4096
DECODE_BATCH = 32
NUM_BLOCKS = 256
BLOCK_SIZE = 256
MAX_BLOCKS = 8
TOTAL = N_PREFILL + DECODE_BATCH  # 4128
N_CORES = 8
MAX_KTILES = 16  # ceil(2047/128)

F32 = mybir.dt.float32
BF16 = mybir.dt.bfloat16
FP16 = mybir.dt.float16
FP8 = mybir.dt.float8e4
EXP = mybir.ActivationFunctionType.Exp

NP_FP8 = ml_dtypes.float8_e4m3fn

_program_cache: dict[bytes, object] = {}


def _decode_order(ctx_lens: np.ndarray) -> list[int]:
    """Seqs sorted by tile count ASC: grouping 4 similar-length seqs
    minimizes sum-of-group-max rounds in the col-tiled decode AV, and
    small-first keeps early prefetch slices small (they must arrive
    before the first decode groups consume them)."""
    ntiles_b = [-(-int(ctx_lens[b]) // 128) for b in range(DECODE_BATCH)]
    return sorted(range(DECODE_BATCH), key=lambda b: (ntiles_b[b], b))


def _build_program(ctx_lens: np.ndarray):
    """Build + finalize the (SPMD-identical) Bass program for one core."""
    nc = bacc.Bacc("TRN2", target_bir_lowering=False)

    # ---- static decode geometry (baked) ----
    ntiles_b = [-(-int(ctx_lens[b]) // 128) for b in range(DECODE_BATCH)]
    order = _decode_order(ctx_lens)
    # decode KV is packed host-side in SORTED order: slot p holds seq
    # order[p], so each group's tiles are contiguous and the prefetch
    # slices arrive in consumption order
    stile_off = [0]
    for p in range(DECODE_BATCH):
        stile_off.append(stile_off[-1] + ntiles_b[order[p]])
    tot_tiles = stile_off[-1]

    qpreT = nc.dram_tensor("qpreT", [NUM_SEQS, 2, HEAD_DIM, 2, SEQLEN], FP16,
                           kind="ExternalInput")
    kpreT = nc.dram_tensor("kpreT", [NUM_SEQS, HEAD_DIM, SEQLEN], FP16,
                           kind="ExternalInput")
    vpre1 = nc.dram_tensor(
        "vpre1", [NUM_SEQS, 128, SEQLEN // 128, HEAD_DIM + 1], FP16,
        kind="ExternalInput")
    qdecT = nc.dram_tensor("qdecT", [HEAD_DIM, DECODE_BATCH * GQA], FP8,
                           kind="ExternalInput")
    kdec = nc.dram_tensor("kdec", [HEAD_DIM, tot_tiles * 128], FP8,
                          kind="ExternalInput")
    vdec = nc.dram_tensor("vdec", [128, tot_tiles, HEAD_DIM + 1], FP8,
                          kind="ExternalInput")
    trimask = nc.dram_tensor("trimask", [128, 2, 128], FP16,
                             kind="ExternalInput")
    tailmask = nc.dram_tensor("tailmask", [128, DECODE_BATCH], F32,
                              kind="ExternalInput")
    # unnormalized prefill out: [s, hp, c, 128 q, hh*4+ml, 129]
    preout = nc.dram_tensor(
        "preout", [NUM_SEQS, 2, 2, 128, 8, HEAD_DIM + 1], FP16,
        kind="ExternalOutput")
    # unnormalized decode out: [8 groups, 128 rows (32i+gqa), 129];
    # group g row 32i+j = seq order[4g+i], gqa head j
    ddec = nc.dram_tensor("ddec", [DECODE_BATCH // 4, 128, HEAD_DIM + 1], F32,
                          kind="ExternalOutput")

    with tile.TileContext(nc) as tc:
        with tc.tile_pool(name="consts", bufs=1) as consts, \
             tc.tile_pool(name="kv8", bufs=1) as kv8_pool:
            tri = consts.tile([128, 2, 128], FP16)
            nc.sync.dma_start(tri, trimask[:, :, :])
            tail_s = consts.tile([128, DECODE_BATCH], F32)
            with tc.tile_wait_until(0.012):
                nc.sync.dma_start(tail_s, tailmask[:, :])
            qdec_s = consts.tile([HEAD_DIM, DECODE_BATCH * GQA], FP8)
            with tc.tile_wait_until(0.012):
                nc.sync.dma_start(qdec_s, qdecT[:, :])
            kp = kv8_pool.tile([HEAD_DIM, tot_tiles * 128], FP8, name="kp")
            vp = kv8_pool.tile([128, tot_tiles, HEAD_DIM + 1], FP8, name="vp")
            kv_prefetch_done = [False]
            # HAM warmup: ~5us of dummy back-to-back matmuls while the real
            # inputs stream in, so the PE clock gate is already at 8/8 when
            # the first real matmul issues (saves ~10us of half-clock start)
            wz = consts.tile([128, 512], FP16, name="wz")
            nc.vector.memset(wz, 0.0)

            # ---------------- prefill + interleaved decode ----------------
            # bufs cover the whole problem: all inputs prefetch at t=0 so no
            # mid-kernel load can be starved by the decode-KV stream
            with tc.tile_pool(name="kT", bufs=4) as kT_pool, \
                 tc.tile_pool(name="v1", bufs=4) as v1_pool, \
                 tc.tile_pool(name="qT", bufs=8) as qT_pool, \
                 tc.tile_pool(name="es", bufs=6) as e_pool, \
                 tc.tile_pool(name="stg", bufs=4) as stg_pool, \
                 tc.tile_pool(name="ed", bufs=2) as ed_pool, \
                 tc.tile_pool(name="dst", bufs=1) as dst_pool, \
                 tc.tile_pool(name="sc", bufs=2, space="PSUM") as s_pool, \
                 tc.tile_pool(name="ot", bufs=1, space="PSUM") as o_pool, \
                 tc.tile_pool(name="dec", bufs=1, space="PSUM") as dec_pool:
                dstage = dst_pool.tile(
                    [128, DECODE_BATCH // 4, HEAD_DIM + 1], F32)

                def emit_kv_prefetch():
                    # decode KV prefetch (9MB): delayed + split into
                    # per-group slices. One huge descriptor starves the
                    # per-descriptor round-robin against the startup-
                    # critical q/k/v loads (first matmul then waits ~30us
                    # on its inputs); small descriptors share fairly and
                    # the wait hint keeps them off the critical window.
                    # slices issue in CONSUMPTION order (the chunk 3..10
                    # group permutation), not packing order
                    for i, g in enumerate(g_of_chunk[k]
                                          for k in sorted(g_of_chunk)):
                        ta = stile_off[g * 4]
                        tb = stile_off[g * 4 + 4]
                        with tc.tile_wait_until(0.018 + 0.005 * i):
                            nc.gpsimd.dma_start(
                                kp[:, ta * 128:tb * 128],
                                kdec[:, ta * 128:tb * 128])
                            nc.gpsimd.dma_start(
                                vp[:, ta:tb, :], vdec[:, ta:tb, :])

                # AV work units are deferred in a global queue and popped
                # lazily, so a chunk's tail AVs (and the decode bursts)
                # overlap the NEXT chunk's first QK+exp emissions: the
                # scalar engine always has exp work queued while the PE
                # grinds AV/decode backlog. ot/stage tiles are allocated
                # inside the first popped unit (not at emission time) so
                # pool address reuse stays in program order.
                avq = []

                def pop_av(n_keep):
                    while len(avq) > n_keep:
                        avq.pop(0)()

                def emit_chunk(s, hp, c, kT, v1, qT, mid_cb, last=False):
                    nj = 4 * (c + 1)
                    ctx = {}

                    def ot_slice(idx):
                        if idx < 3:
                            return ctx["otA"][:, idx, :]
                        if idx < 6:
                            return ctx["otB"][:, idx - 3, :]
                        return ctx["otC"][:, idx - 6, :]

                    def emit_av(j, e, cmap=None):
                        if not ctx:
                            # 8 otile slots (hh*4+ml) packed 3/3/2 per bank
                            ctx["otA"] = o_pool.tile(
                                [128, 3, HEAD_DIM + 1], F32, name="otA",
                                tag="otA")
                            ctx["otB"] = o_pool.tile(
                                [128, 3, HEAD_DIM + 1], F32, name="otB",
                                tag="otB")
                            ctx["otC"] = o_pool.tile(
                                [128, 2, HEAD_DIM + 1], F32, name="otC",
                                tag="otC")
                            ctx["stage"] = stg_pool.tile(
                                [128, 8, HEAD_DIM + 1], FP16, name="stage")
                        # pair up evacuations that share a PSUM tile: CAST
                        # both slots once the later one stops (5 DVE
                        # ops/chunk)
                        evac_at = {1: ("otA", 0, 0, 2), 2: ("otA", 2, 2, 3),
                                   3: ("otB", 0, 3, 4), 5: ("otB", 1, 4, 6),
                                   7: ("otC", 0, 6, 8)}
                        ml0 = j - 4 * c if j > 4 * c else 0
                        for hh in range(2):
                            for ml in range(ml0, 4):
                                idx = hh * 4 + ml
                                co = cmap[ml] if cmap else ml * 128
                                # start=True clears has_written for the WHOLE
                                # PSUM bank, so only the first group written to
                                # each bank (idx 0/3/6) may set it; the other
                                # groups' first writes land on cleared bits and
                                # overwrite anyway.
                                nc.tensor.matmul(
                                    ot_slice(idx),
                                    e[:, hh, co:co + 128],
                                    v1[:, j, :],
                                    start=(j == 0 and idx in (0, 3, 6)),
                                    stop=(j == 4 * c + ml),
                                    skip_group_check=True)
                                if j == 4 * c + ml and idx in evac_at:
                                    tname, k0, i0, i1 = evac_at[idx]
                                    nc.vector.tensor_copy(
                                        ctx["stage"][:, i0:i1, :],
                                        ctx[tname][:, k0:k0 + (i1 - i0), :])
                        if last and j == nj - 2:
                            # final chunk: ship finished slots early on the
                            # otherwise-idle sync queue to shorten the
                            # end-of-kernel DMA drain
                            nc.sync.dma_start(
                                preout[s, hp, c][:, 0:3, :],
                                ctx["stage"][:, 0:3, :])
                        if j == nj - 1:
                            if last:
                                nc.sync.dma_start(
                                    preout[s, hp, c][:, 3:8, :],
                                    ctx["stage"][:, 3:8, :])
                            else:
                                nc.gpsimd.dma_start(
                                    preout[s, hp, c], ctx["stage"])
                            if not kv_prefetch_done[0]:
                                kv_prefetch_done[0] = True
                                emit_kv_prefetch()

                    def emit_j(j):
                        off = 128 * (j - 4 * c) if j > 4 * c else 0
                        sc = s_pool.tile([128, 2, 512], F32, name="sc")
                        for hh in range(2):
                            nc.tensor.matmul(
                                sc[:, hh, off:],
                                kT[:, j * 128:(j + 1) * 128],
                                qT[:, hh, c * 512 + off:(c + 1) * 512],
                                start=True, stop=True)
                        e = e_pool.tile([128, 2, 512], FP16, name="e")
                        nc.scalar.activation(
                            e[:, :, off:], sc[:, :, off:], EXP, scale=SCALE)
                        if j >= 4 * c:
                            # one DVE op masks both heads (tri is duplicated
                            # along the hh axis host-side)
                            nc.vector.tensor_mul(
                                e[:, :, off:off + 128],
                                e[:, :, off:off + 128],
                                tri)
                        avq.append(lambda jj=j, ee=e: emit_av(jj, ee))

                    def emit_pack():
                        # last two diagonal k-tiles (widths 256+128) packed
                        # into ONE sc slot at stacked column offsets: one
                        # ACTIVATE instead of two (each carries a ~293ns
                        # fixed cost); the AV reads remap via cmap
                        ja, jb = nj - 2, nj - 1
                        sc = s_pool.tile([128, 2, 512], F32, name="sc")
                        for hh in range(2):
                            nc.tensor.matmul(
                                sc[:, hh, 0:256],
                                kT[:, ja * 128:(ja + 1) * 128],
                                qT[:, hh, c * 512 + 256:(c + 1) * 512],
                                start=True, stop=True)
                        for hh in range(2):
                            nc.tensor.matmul(
                                sc[:, hh, 256:384],
                                kT[:, jb * 128:(jb + 1) * 128],
                                qT[:, hh, c * 512 + 384:(c + 1) * 512],
                                start=True, stop=True)
                        e = e_pool.tile([128, 2, 512], FP16, name="e")
                        nc.scalar.activation(
                            e[:, :, 0:384], sc[:, :, 0:384], EXP, scale=SCALE)
                        nc.vector.tensor_mul(
                            e[:, :, 0:128], e[:, :, 0:128], tri)
                        nc.vector.tensor_mul(
                            e[:, :, 256:384], e[:, :, 256:384], tri)
                        avq.append(
                            lambda ee=e: emit_av(ja, ee, {2: 0, 3: 128}))
                        avq.append(lambda ee=e: emit_av(jb, ee, {3: 256}))

                    # 3 exps queued ahead of the prev-chunk AV backlog
                    # (e_pool bufs=6 is sized exactly for this pre-emission
                    # depth + the 4-unit chunk carry-over -- do not raise
                    # either without resizing the pool). Decode slices drip
                    # in between the exp emissions so ScalarE never idles
                    # behind a monolithic decode burst.
                    slices = mid_cb()

                    def take_slice():
                        if slices:
                            slices.pop(0)()
                    nsteps = nj - 1
                    for si in range(nsteps):
                        if si >= 3:
                            pop_av(2)
                            take_slice()
                        if si == nsteps - 1:
                            emit_pack()
                        else:
                            emit_j(si)
                        if si < 3:
                            take_slice()
                    if nsteps <= 3:
                        pop_av(4)
                    while slices:
                        slices.pop(0)()

                # short HAM-bridge warmup: just enough back-to-back matmuls
                # to keep the PE activity window busy until the first real
                # QK's inputs land (~8us); more only delays real work (each
                # cold N=512 matmul burns 427ns of runway)
                scw = dec_pool.tile([128, 512], F32, name="dec")
                for _ in range(6):
                    nc.tensor.matmul(scw, wz[:, 0:128], wz,
                                     start=True, stop=True,
                                     skip_group_check=True)

                NG = 4  # decode seqs per group (one per PE column-group)
                SDW = NG * 4 * MAX_KTILES  # score-col capacity per group
                grp_seqs = [order[g * NG:(g + 1) * NG]
                            for g in range(DECODE_BATCH // NG)]
                # groups ride chunks 4..11; c1 chunks (odd k) have 8 slice
                # positions vs c0's 4, so the big groups (4..7, sizes are
                # ascending) go to c1 chunks and the small ones to c0
                g_of_chunk = {4: 0, 5: 4, 6: 1, 7: 5, 8: 2, 9: 6, 10: 3,
                              11: 7}
                # compact per-group score layout: seq i's cols at
                # [coff[i], coff[i]+4*nt) -- exp only touches real scores
                grp_coff = []
                for seqs in grp_seqs:
                    coff = [0]
                    for b in seqs:
                        coff.append(coff[-1] + 4 * ntiles_b[b])
                    grp_coff.append(coff)

                def emit_dec_qk_parts(g):
                    # sd (cols 0:W) + od (cols 256:385) share one PSUM bank
                    # via flat offsets (PSUM slots pad to whole banks, so two
                    # tags won't pack). Emission is split into <=16-matmul
                    # slices so the prefill exps interleave and ScalarE never
                    # starves behind one long decode burst.
                    seqs, coff = grp_seqs[g], grp_coff[g]
                    W = coff[-1]
                    state = {}
                    mms = [(i, t) for i in range(NG)
                           for t in range(ntiles_b[seqs[i]])]
                    parts = []
                    for lo in range(0, len(mms), 16):
                        sub = mms[lo:lo + 16]

                        def qk_slice(sub=sub, first=(lo == 0)):
                            if first:
                                state["dec"] = dec_pool.tile(
                                    [128, 512], F32, name="dec")
                                nc.vector.memset(state["dec"][:, 0:W], 0.0)
                            for i, t in sub:
                                b = seqs[i]
                                gt = stile_off[g * NG + i] + t
                                o = coff[i] + 4 * t
                                nc.tensor.matmul(
                                    state["dec"][:, o:o + 4],
                                    kp[:, gt * 128:(gt + 1) * 128],
                                    qdec_s[:, 4 * b:4 * b + 4],
                                    start=True, stop=True,
                                    skip_group_check=True)
                        parts.append(qk_slice)

                    def qk_exp():
                        ed = ed_pool.tile([128, SDW], FP16, name="ed")
                        nc.scalar.activation(ed[:, 0:W], state["dec"][:, 0:W],
                                             EXP, scale=SCALE)
                        dec_pending[0] = (g, state["dec"], ed)
                    parts.append(qk_exp)
                    return parts

                def emit_dec_av_parts(g, dec, ed):
                    seqs, coff = grp_seqs[g], grp_coff[g]
                    od = dec[:, SDW:SDW + HEAD_DIM + 1]
                    # col-tiled AV: the 4 seqs' [128tok x 4q] stationaries sit
                    # in distinct 32-col groups of the PE array, so each
                    # round's 4 matmuls stream their V tiles CONCURRENTLY
                    # (~3.3x on this weight-bound stage).
                    # start=True clears has_written for the whole bank, so
                    # only the first matmul of round 0 sets it; the other
                    # seqs' first writes land on cleared bits and overwrite.
                    max_nt = max(ntiles_b[b] for b in seqs)

                    def av_rounds(t0, t1):
                        for t in range(t0, t1):
                            for i in range(NG):
                                b = seqs[i]
                                nt = ntiles_b[b]
                                if t >= nt:
                                    continue
                                gt = stile_off[g * NG + i] + t
                                o = coff[i] + 4 * t
                                nc.tensor.matmul(
                                    od[32 * i:32 * i + GQA, :],
                                    ed[:, o:o + 4],
                                    vp[:, gt, :],
                                    start=(t == 0 and i == 0),
                                    stop=(t == nt - 1),
                                    skip_group_check=True,
                                    tile_position=(0, 32 * i))

                    def av_a():
                        for i in range(NG):
                            b = seqs[i]
                            nt = ntiles_b[b]
                            rem = int(ctx_lens[b]) - 128 * (nt - 1)
                            if rem < 128:
                                o = coff[i] + 4 * (nt - 1)
                                nc.vector.tensor_scalar_mul(
                                    ed[:, o:o + 4], ed[:, o:o + 4],
                                    tail_s[:, b:b + 1])
                        av_rounds(0, (max_nt + 1) // 2)

                    def av_b():
                        av_rounds((max_nt + 1) // 2, max_nt)
                        nc.vector.tensor_copy(dstage[:, g, :], od[:, :])
                        nc.gpsimd.dma_start(ddec[g], dstage[:, g, :])
                    return [av_a, av_b]

                # decode groups ride the back half of the prefill chunk list:
                # a full prefill chunk's matmuls sit between each group's QK
                # and its AV, hiding the exp latency, and the decode work
                # fills the PE slack of the ScalarE-paced prefill chunks.
                n_groups = DECODE_BATCH // NG
                chunk_no = [0]
                dec_pending = [None]

                def decode_slices():
                    k = chunk_no[0]
                    chunk_no[0] += 1
                    # groups ride chunks 3..10: clear of both the
                    # startup-critical loads and the final-chunk tail
                    slices = []
                    if dec_pending[0] is not None:
                        slices += emit_dec_av_parts(*dec_pending[0])
                        dec_pending[0] = None
                    if k in g_of_chunk:
                        slices += emit_dec_qk_parts(g_of_chunk[k])
                    return slices

                for s in range(NUM_SEQS):
                    kT = kT_pool.tile([128, SEQLEN], FP16, name="kT")
                    # one descriptor per tensor: the sync engine issues
                    # descriptors at ~600ns each, so fewer+larger loads get
                    # chunk-0/1's inputs resident sooner (the HAM warmup
                    # bridges the slightly longer first transfer)
                    nc.sync.dma_start(kT, kpreT[s])
                    for hp in range(2):
                        qT = qT_pool.tile([128, 2, SEQLEN], FP16, name="qT")
                        for hh in range(2):
                            nc.sync.dma_start(
                                qT[:, hh, :], qpreT[s, hp, :, hh, :])
                        if hp == 0:
                            v1 = v1_pool.tile(
                                [128, SEQLEN // 128, HEAD_DIM + 1], FP16,
                                name="v1")
                            nc.sync.dma_start(v1, vpre1[s])
                        # the very last chunk runs c=0 (4 k-tiles): shorter
                        # AV+evac+DMA tail after the final exp
                        corder = (1, 0) if (s == NUM_SEQS - 1 and hp == 1) \
                            else (0, 1)
                        for c in corder:
                            emit_chunk(s, hp, c, kT, v1, qT, decode_slices,
                                       last=(s == NUM_SEQS - 1 and hp == 1))
                if dec_pending[0] is not None:
                    for part in emit_dec_av_parts(*dec_pending[0]):
                        part()
                    dec_pending[0] = None
                pop_av(0)

    nc.finalize()
    return nc


def kernel(q, k, v, k_cache, v_cache, slot_mapping, context_lens,
           decode_block_tables, **_unused):
    q = np.asarray(q, dtype=np.float32)
    k = np.asarray(k, dtype=np.float32)
    v = np.asarray(v, dtype=np.float32)
    k_cache = np.asarray(k_cache, dtype=np.float32)
    v_cache = np.asarray(v_cache, dtype=np.float32)
    slot_mapping = np.asarray(slot_mapping)
    context_lens = np.asarray(context_lens)
    decode_block_tables = np.asarray(decode_block_tables)

    # ---- host prep: apply the kv-cache scatter (the reference's
    # _store_kvcache) so decode reads the updated cache ----
    kc = k_cache.reshape(NUM_BLOCKS * BLOCK_SIZE, NUM_KV_HEADS, HEAD_DIM).copy()
    vc = v_cache.reshape(NUM_BLOCKS * BLOCK_SIZE, NUM_KV_HEADS, HEAD_DIM).copy()
    kc[slot_mapping] = k
    vc[slot_mapping] = v
    kc = kc.reshape(NUM_BLOCKS, BLOCK_SIZE, NUM_KV_HEADS, HEAD_DIM)
    vc = vc.reshape(NUM_BLOCKS, BLOCK_SIZE, NUM_KV_HEADS, HEAD_DIM)

    qpre = q[:N_PREFILL].reshape(NUM_SEQS, SEQLEN, NUM_HEADS, HEAD_DIM)
    kpre = k[:N_PREFILL].reshape(NUM_SEQS, SEQLEN, NUM_KV_HEADS, HEAD_DIM)
    vpre = v[:N_PREFILL].reshape(NUM_SEQS, SEQLEN, NUM_KV_HEADS, HEAD_DIM)
    qdec = q[N_PREFILL:]  # [32, 32, 128]

    ones_pre = np.ones((NUM_SEQS, SEQLEN, 1), np.float32)
    trimask = np.broadcast_to(
        (np.arange(128)[:, None, None] <= np.arange(128)[None, None, :]),
        (128, 2, 128)).astype(np.float16)
    ntiles_b = (-(-context_lens.astype(np.int64) // 128)).astype(np.int64)
    rem_b = context_lens.astype(np.int64) - 128 * (ntiles_b - 1)
    tailmask = (np.arange(128)[:, None] < rem_b[None, :]).astype(np.float32)
    nblocks_b = -(-context_lens.astype(np.int64) // BLOCK_SIZE)
    tot_tiles = int(ntiles_b.sum())
    order = _decode_order(context_lens)

    in_maps = []
    for c in range(N_CORES):
        h0 = c * GQA
        # [s, hp, d, hh, q]
        qpreT = np.ascontiguousarray(
            qpre[:, :, h0:h0 + GQA, :]
            .reshape(NUM_SEQS, SEQLEN, 2, 2, HEAD_DIM)
            .transpose(0, 2, 4, 3, 1)).astype(np.float16)
        kpreT = np.ascontiguousarray(
            kpre[:, :, c, :].transpose(0, 2, 1)).astype(np.float16)
        vpre1 = np.ascontiguousarray(
            np.concatenate([vpre[:, :, c, :], ones_pre], axis=2)
            .reshape(NUM_SEQS, SEQLEN // 128, 128, HEAD_DIM + 1)
            .transpose(0, 2, 1, 3)).astype(np.float16)
        qdecT = np.ascontiguousarray(
            qdec[:, h0:h0 + GQA, :].transpose(2, 0, 1)
            .reshape(HEAD_DIM, DECODE_BATCH * GQA)).astype(NP_FP8)
        # decode pages packed at 128-token granularity, in sorted-group
        # order so prefetch slices arrive in consumption order
        kparts, vparts = [], []
        for b in order:
            nb = int(nblocks_b[b])
            ntok = int(ntiles_b[b]) * 128
            kg = kc[decode_block_tables[b, :nb], :, c, :] \
                .reshape(nb * BLOCK_SIZE, HEAD_DIM)[:ntok]
            vg = vc[decode_block_tables[b, :nb], :, c, :] \
                .reshape(nb * BLOCK_SIZE, HEAD_DIM)[:ntok]
            kparts.append(kg)
            vparts.append(
                np.concatenate([vg, np.ones((ntok, 1), np.float32)], axis=1))
        kdec = np.ascontiguousarray(
            np.concatenate(kparts, axis=0).T).astype(NP_FP8)
        vdec = np.ascontiguousarray(
            np.concatenate(vparts, axis=0)
            .reshape(tot_tiles, 128, HEAD_DIM + 1)
            .transpose(1, 0, 2)).astype(NP_FP8)
        in_maps.append({
            "qpreT": qpreT, "kpreT": kpreT, "vpre1": vpre1,
            "qdecT": qdecT, "kdec": kdec, "vdec": vdec, "trimask": trimask,
            "tailmask": tailmask,
        })

    key = (np.ascontiguousarray(context_lens).tobytes()
           + np.ascontiguousarray(decode_block_tables).tobytes())
    nc = _program_cache.get(key)
    if nc is None:
        nc = _build_program(context_lens)
        _program_cache[key] = nc

    res = run_bass_kernel_spmd(nc, in_maps, core_ids=list(range(N_CORES)))

    out = np.empty((TOTAL, NUM_HEADS, HEAD_DIM), np.float32)
    for c in range(N_CORES):
        # prefill: [s, hp, ch, qp, hh*4+ml, 129] -> [s, ch, ml, qp, hp, hh, d]
        po = res.results[c]["preout"].astype(np.float32).reshape(
            NUM_SEQS, 2, 2, 128, 2, 4, HEAD_DIM + 1)
        po = po.transpose(0, 2, 5, 3, 1, 4, 6).reshape(
            N_PREFILL, GQA, HEAD_DIM + 1)
        out[:N_PREFILL, c * GQA:(c + 1) * GQA, :] = \
            po[:, :, :HEAD_DIM] / po[:, :, HEAD_DIM:]
        # decode: [8 groups, 128 rows (32i+gqa head), 129]; group g row
        # 32i+j = seq order[4g+i], head j
        dd = res.results[c]["ddec"].reshape(
            DECODE_BATCH // 4, 4, 32, HEAD_DIM + 1)[:, :, :GQA, :]
        dd = (dd[..., :HEAD_DIM] / dd[..., HEAD_DIM:]).reshape(
            DECODE_BATCH, GQA, HEAD_DIM)
        out[N_PREFILL + np.asarray(order), c * GQA:(c + 1) * GQA, :] = dd
    return out



# revision 49
# speedup vs baseline: 1.0091x; 1.0091x over previous
"""Paged GQA attention (prefill + decode) for 8 Trainium2 NeuronCores.

Sharding: tensor-parallel over kv-heads. Core c owns kv-head c and its 4 GQA
query heads. Block tables / context lens are baked into the program (compiled
per call), so all control flow and gather addresses are static.

v2 design (vs the 219us baseline):
  - all prefill operands fp16: halves input DMA and enables FWL weight loads
    (fp32r stationaries can't use FWL, serializing 107ns LDWEIGHTS per matmul)
  - QK matmuls causally column-sliced (25% fewer PE columns)
  - exp merged across 2 query heads per instruction (each ACTIVATE carries a
    ~352-cycle fixed overhead; merging halves the count) with causal column
    skipping via 3D strided APs
  - outputs written unnormalized (with the ones-column row sums) as fp16;
    the division happens on host: frees ~90us of VectorE work and halves
    output DMA
  - decode uses fp8e4m3 KV packed at 128-token granularity, loaded in 2 large
    DMAs at program start (instead of 64 small ones), and runs as a separate
    phase at the end so prefill PSUM pools can be released and reused
"""

import sys

if "/opt/trn_rl_repo" not in sys.path:
    sys.path.insert(0, "/opt/trn_rl_repo")

import numpy as np
import ml_dtypes

import concourse.bass as bass  # noqa: F401  (registers AP machinery)
import concourse.mybir as mybir
import concourse.tile as tile
from concourse import bacc
from concourse.bass_utils import run_bass_kernel_spmd

NUM_HEADS = 32
NUM_KV_HEADS = 8
HEAD_DIM = 128
GQA = NUM_HEADS // NUM_KV_HEADS  # 4
SCALE = 0.08838834764831845
NUM_SEQS = 4
SEQLEN = 1024
N_PREFILL = NUM_SEQS * SEQLEN  # 4096
DECODE_BATCH = 32
NUM_BLOCKS = 256
BLOCK_SIZE = 256
MAX_BLOCKS = 8
TOTAL = N_PREFILL + DECODE_BATCH  # 4128
N_CORES = 8
MAX_KTILES = 16  # ceil(2047/128)

F32 = mybir.dt.float32
BF16 = mybir.dt.bfloat16
FP16 = mybir.dt.float16
FP8 = mybir.dt.float8e4
EXP = mybir.ActivationFunctionType.Exp

NP_FP8 = ml_dtypes.float8_e4m3fn

_program_cache: dict[bytes, object] = {}


def _decode_order(ctx_lens: np.ndarray) -> list[int]:
    """Seqs sorted by tile count ASC: grouping 4 similar-length seqs
    minimizes sum-of-group-max rounds in the col-tiled decode AV, and
    small-first keeps early prefetch slices small (they must arrive
    before the first decode groups consume them)."""
    ntiles_b = [-(-int(ctx_lens[b]) // 128) for b in range(DECODE_BATCH)]
    return sorted(range(DECODE_BATCH), key=lambda b: (ntiles_b[b], b))


def _build_program(ctx_lens: np.ndarray):
    """Build + finalize the (SPMD-identical) Bass program for one core."""
    nc = bacc.Bacc("TRN2", target_bir_lowering=False)

    # ---- static decode geometry (baked) ----
    ntiles_b = [-(-int(ctx_lens[b]) // 128) for b in range(DECODE_BATCH)]
    order = _decode_order(ctx_lens)
    # decode KV is packed host-side in SORTED order: slot p holds seq
    # order[p], so each group's tiles are contiguous and the prefetch
    # slices arrive in consumption order
    stile_off = [0]
    for p in range(DECODE_BATCH):
        stile_off.append(stile_off[-1] + ntiles_b[order[p]])
    tot_tiles = stile_off[-1]

    qpreT = nc.dram_tensor("qpreT", [NUM_SEQS, 2, HEAD_DIM, 2, SEQLEN], FP16,
                           kind="ExternalInput")
    kpreT = nc.dram_tensor("kpreT", [NUM_SEQS, HEAD_DIM, SEQLEN], FP16,
                           kind="ExternalInput")
    vpre1 = nc.dram_tensor(
        "vpre1", [NUM_SEQS, 128, SEQLEN // 128, HEAD_DIM + 1], FP16,
        kind="ExternalInput")
    qdecT = nc.dram_tensor("qdecT", [HEAD_DIM, DECODE_BATCH * GQA], FP8,
                           kind="ExternalInput")
    kdec = nc.dram_tensor("kdec", [HEAD_DIM, tot_tiles * 128], FP8,
                          kind="ExternalInput")
    vdec = nc.dram_tensor("vdec", [128, tot_tiles, HEAD_DIM + 1], FP8,
                          kind="ExternalInput")
    trimask = nc.dram_tensor("trimask", [128, 2, 128], FP16,
                             kind="ExternalInput")
    tailmask = nc.dram_tensor("tailmask", [128, DECODE_BATCH], F32,
                              kind="ExternalInput")
    # unnormalized prefill out: [s, hp, c, 128 q, hh*4+ml, 129]
    preout = nc.dram_tensor(
        "preout", [NUM_SEQS, 2, 2, 128, 8, HEAD_DIM + 1], FP16,
        kind="ExternalOutput")
    # unnormalized decode out: [8 groups, 128 rows (32i+gqa), 129];
    # group g row 32i+j = seq order[4g+i], gqa head j
    ddec = nc.dram_tensor("ddec", [DECODE_BATCH // 4, 128, HEAD_DIM + 1], F32,
                          kind="ExternalOutput")

    with tile.TileContext(nc) as tc:
        with tc.tile_pool(name="consts", bufs=1) as consts, \
             tc.tile_pool(name="kv8", bufs=1) as kv8_pool:
            tri = consts.tile([128, 2, 128], FP16)
            nc.sync.dma_start(tri, trimask[:, :, :])
            # tail_s/qdec_s loads are emitted at s==2 in the seq loop: the
            # sync queue is IN-ORDER, so a wait-hinted load here would stall
            # every input load behind it until the hint passes (observed as
            # chunk-0/1 QKs waiting to ~13us and a HAM re-throttle)
            tail_s = consts.tile([128, DECODE_BATCH], F32)
            qdec_s = consts.tile([HEAD_DIM, DECODE_BATCH * GQA], FP8)
            kp = kv8_pool.tile([HEAD_DIM, tot_tiles * 128], FP8, name="kp")
            vp = kv8_pool.tile([128, tot_tiles, HEAD_DIM + 1], FP8, name="vp")
            kv_prefetch_done = [False]
            # HAM warmup: ~5us of dummy back-to-back matmuls while the real
            # inputs stream in, so the PE clock gate is already at 8/8 when
            # the first real matmul issues (saves ~10us of half-clock start)
            wz = consts.tile([128, 512], FP16, name="wz")
            nc.vector.memset(wz, 0.0)

            # ---------------- prefill + interleaved decode ----------------
            # bufs cover the whole problem: all inputs prefetch at t=0 so no
            # mid-kernel load can be starved by the decode-KV stream
            with tc.tile_pool(name="kT", bufs=4) as kT_pool, \
                 tc.tile_pool(name="v1", bufs=4) as v1_pool, \
                 tc.tile_pool(name="qT", bufs=8) as qT_pool, \
                 tc.tile_pool(name="es", bufs=6) as e_pool, \
                 tc.tile_pool(name="stg", bufs=4) as stg_pool, \
                 tc.tile_pool(name="ed", bufs=2) as ed_pool, \
                 tc.tile_pool(name="dst", bufs=1) as dst_pool, \
                 tc.tile_pool(name="sc", bufs=2, space="PSUM") as s_pool, \
                 tc.tile_pool(name="ot", bufs=1, space="PSUM") as o_pool, \
                 tc.tile_pool(name="dec", bufs=1, space="PSUM") as dec_pool:
                dstage = dst_pool.tile(
                    [128, DECODE_BATCH // 4, HEAD_DIM + 1], F32)

                def emit_kv_prefetch():
                    # decode KV prefetch (9MB): delayed + split into
                    # per-group slices. One huge descriptor starves the
                    # per-descriptor round-robin against the startup-
                    # critical q/k/v loads (first matmul then waits ~30us
                    # on its inputs); small descriptors share fairly and
                    # the wait hint keeps them off the critical window.
                    # slices issue in CONSUMPTION order (the chunk 3..10
                    # group permutation), not packing order
                    for i, g in enumerate(g_of_chunk[k]
                                          for k in sorted(g_of_chunk)):
                        ta = stile_off[g * 4]
                        tb = stile_off[g * 4 + 4]
                        with tc.tile_wait_until(0.018 + 0.005 * i):
                            nc.gpsimd.dma_start(
                                kp[:, ta * 128:tb * 128],
                                kdec[:, ta * 128:tb * 128])
                            nc.gpsimd.dma_start(
                                vp[:, ta:tb, :], vdec[:, ta:tb, :])

                # AV work units are deferred in a global queue and popped
                # lazily, so a chunk's tail AVs (and the decode bursts)
                # overlap the NEXT chunk's first QK+exp emissions: the
                # scalar engine always has exp work queued while the PE
                # grinds AV/decode backlog. ot/stage tiles are allocated
                # inside the first popped unit (not at emission time) so
                # pool address reuse stays in program order.
                avq = []

                def pop_av(n_keep):
                    while len(avq) > n_keep:
                        avq.pop(0)()

                def emit_chunk(s, hp, c, kT, v1, qT, mid_cb, last=False):
                    nj = 4 * (c + 1)
                    ctx = {}

                    def ot_slice(idx):
                        if idx < 3:
                            return ctx["otA"][:, idx, :]
                        if idx < 6:
                            return ctx["otB"][:, idx - 3, :]
                        return ctx["otC"][:, idx - 6, :]

                    def emit_av(j, e, cmap=None):
                        if not ctx:
                            # 8 otile slots (hh*4+ml) packed 3/3/2 per bank
                            ctx["otA"] = o_pool.tile(
                                [128, 3, HEAD_DIM + 1], F32, name="otA",
                                tag="otA")
                            ctx["otB"] = o_pool.tile(
                                [128, 3, HEAD_DIM + 1], F32, name="otB",
                                tag="otB")
                            ctx["otC"] = o_pool.tile(
                                [128, 2, HEAD_DIM + 1], F32, name="otC",
                                tag="otC")
                            ctx["stage"] = stg_pool.tile(
                                [128, 8, HEAD_DIM + 1], FP16, name="stage")
                        # pair up evacuations that share a PSUM tile: CAST
                        # both slots once the later one stops (5 DVE
                        # ops/chunk)
                        evac_at = {1: ("otA", 0, 0, 2), 2: ("otA", 2, 2, 3),
                                   3: ("otB", 0, 3, 4), 5: ("otB", 1, 4, 6),
                                   7: ("otC", 0, 6, 8)}
                        ml0 = j - 4 * c if j > 4 * c else 0
                        for hh in range(2):
                            for ml in range(ml0, 4):
                                idx = hh * 4 + ml
                                co = cmap[ml] if cmap else ml * 128
                                # start=True clears has_written for the WHOLE
                                # PSUM bank, so only the first group written to
                                # each bank (idx 0/3/6) may set it; the other
                                # groups' first writes land on cleared bits and
                                # overwrite anyway.
                                nc.tensor.matmul(
                                    ot_slice(idx),
                                    e[:, hh, co:co + 128],
                                    v1[:, j, :],
                                    start=(j == 0 and idx in (0, 3, 6)),
                                    stop=(j == 4 * c + ml),
                                    skip_group_check=True)
                                if j == 4 * c + ml and idx in evac_at:
                                    tname, k0, i0, i1 = evac_at[idx]
                                    nc.vector.tensor_copy(
                                        ctx["stage"][:, i0:i1, :],
                                        ctx[tname][:, k0:k0 + (i1 - i0), :])
                        if last and j == nj - 2:
                            # final chunk: ship finished slots early on the
                            # otherwise-idle sync queue to shorten the
                            # end-of-kernel DMA drain
                            nc.sync.dma_start(
                                preout[s, hp, c][:, 0:3, :],
                                ctx["stage"][:, 0:3, :])
                        if j == nj - 1:
                            if last:
                                nc.sync.dma_start(
                                    preout[s, hp, c][:, 3:8, :],
                                    ctx["stage"][:, 3:8, :])
                            else:
                                nc.gpsimd.dma_start(
                                    preout[s, hp, c], ctx["stage"])
                            if not kv_prefetch_done[0]:
                                kv_prefetch_done[0] = True
                                emit_kv_prefetch()

                    def emit_j(j):
                        off = 128 * (j - 4 * c) if j > 4 * c else 0
                        sc = s_pool.tile([128, 2, 512], F32, name="sc")
                        for hh in range(2):
                            nc.tensor.matmul(
                                sc[:, hh, off:],
                                kT[:, j * 128:(j + 1) * 128],
                                qT[:, hh, c * 512 + off:(c + 1) * 512],
                                start=True, stop=True)
                        e = e_pool.tile([128, 2, 512], FP16, name="e")
                        nc.scalar.activation(
                            e[:, :, off:], sc[:, :, off:], EXP, scale=SCALE)
                        if j >= 4 * c:
                            # one DVE op masks both heads (tri is duplicated
                            # along the hh axis host-side)
                            nc.vector.tensor_mul(
                                e[:, :, off:off + 128],
                                e[:, :, off:off + 128],
                                tri)
                        avq.append(lambda jj=j, ee=e: emit_av(jj, ee))

                    def emit_pack():
                        # last two diagonal k-tiles (widths 256+128) packed
                        # into ONE sc slot at stacked column offsets: one
                        # ACTIVATE instead of two (each carries a ~293ns
                        # fixed cost); the AV reads remap via cmap
                        ja, jb = nj - 2, nj - 1
                        sc = s_pool.tile([128, 2, 512], F32, name="sc")
                        for hh in range(2):
                            nc.tensor.matmul(
                                sc[:, hh, 0:256],
                                kT[:, ja * 128:(ja + 1) * 128],
                                qT[:, hh, c * 512 + 256:(c + 1) * 512],
                                start=True, stop=True)
                        for hh in range(2):
                            nc.tensor.matmul(
                                sc[:, hh, 256:384],
                                kT[:, jb * 128:(jb + 1) * 128],
                                qT[:, hh, c * 512 + 384:(c + 1) * 512],
                                start=True, stop=True)
                        e = e_pool.tile([128, 2, 512], FP16, name="e")
                        nc.scalar.activation(
                            e[:, :, 0:384], sc[:, :, 0:384], EXP, scale=SCALE)
                        nc.vector.tensor_mul(
                            e[:, :, 0:128], e[:, :, 0:128], tri)
                        nc.vector.tensor_mul(
                            e[:, :, 256:384], e[:, :, 256:384], tri)
                        avq.append(
                            lambda ee=e: emit_av(ja, ee, {2: 0, 3: 128}))
                        avq.append(lambda ee=e: emit_av(jb, ee, {3: 256}))

                    # 3 exps queued ahead of the prev-chunk AV backlog
                    # (e_pool bufs=6 is sized exactly for this pre-emission
                    # depth + the 4-unit chunk carry-over -- do not raise
                    # either without resizing the pool). Decode slices drip
                    # in between the exp emissions so ScalarE never idles
                    # behind a monolithic decode burst.
                    slices = mid_cb()

                    def take_slice():
                        if slices:
                            slices.pop(0)()
                    nsteps = nj - 1
                    for si in range(nsteps):
                        if si >= 3:
                            pop_av(2)
                            take_slice()
                        if si == nsteps - 1:
                            emit_pack()
                        else:
                            emit_j(si)
                        if si < 3:
                            take_slice()
                        if chunk_no[0] <= 3 and si < 2:
                            # HAM bridge: one dummy matmul after the early
                            # steps keeps the PE activity window busy across
                            # any residual input-DMA wait, so the clock gate
                            # doesn't re-throttle during chunks 0-2
                            nc.tensor.matmul(scw, wz[:, 0:128], wz,
                                             start=True, stop=True,
                                             skip_group_check=True)
                    if nsteps <= 3:
                        pop_av(4)
                    while slices:
                        slices.pop(0)()

                # short HAM-bridge warmup: just enough back-to-back matmuls
                # to keep the PE activity window busy until the first real
                # QK's inputs land (~8us); more only delays real work (each
                # cold N=512 matmul burns 427ns of runway)
                scw = dec_pool.tile([128, 512], F32, name="dec")
                for _ in range(6):
                    nc.tensor.matmul(scw, wz[:, 0:128], wz,
                                     start=True, stop=True,
                                     skip_group_check=True)

                NG = 4  # decode seqs per group (one per PE column-group)
                SDW = NG * 4 * MAX_KTILES  # score-col capacity per group
                grp_seqs = [order[g * NG:(g + 1) * NG]
                            for g in range(DECODE_BATCH // NG)]
                # groups ride chunks 4..11; c1 chunks (odd k) have 8 slice
                # positions vs c0's 4, so the big groups (4..7, sizes are
                # ascending) go to c1 chunks and the small ones to c0
                g_of_chunk = {4: 0, 5: 4, 6: 1, 7: 5, 8: 2, 9: 6, 10: 3,
                              11: 7}
                # compact per-group score layout: seq i's cols at
                # [coff[i], coff[i]+4*nt) -- exp only touches real scores
                grp_coff = []
                for seqs in grp_seqs:
                    coff = [0]
                    for b in seqs:
                        coff.append(coff[-1] + 4 * ntiles_b[b])
                    grp_coff.append(coff)

                def emit_dec_qk_parts(g):
                    # sd (cols 0:W) + od (cols 256:385) share one PSUM bank
                    # via flat offsets (PSUM slots pad to whole banks, so two
                    # tags won't pack). Emission is split into <=16-matmul
                    # slices so the prefill exps interleave and ScalarE never
                    # starves behind one long decode burst.
                    seqs, coff = grp_seqs[g], grp_coff[g]
                    W = coff[-1]
                    state = {}
                    mms = [(i, t) for i in range(NG)
                           for t in range(ntiles_b[seqs[i]])]
                    parts = []
                    for lo in range(0, len(mms), 16):
                        sub = mms[lo:lo + 16]

                        def qk_slice(sub=sub, first=(lo == 0)):
                            if first:
                                state["dec"] = dec_pool.tile(
                                    [128, 512], F32, name="dec")
                                nc.vector.memset(state["dec"][:, 0:W], 0.0)
                            for i, t in sub:
                                b = seqs[i]
                                gt = stile_off[g * NG + i] + t
                                o = coff[i] + 4 * t
                                nc.tensor.matmul(
                                    state["dec"][:, o:o + 4],
                                    kp[:, gt * 128:(gt + 1) * 128],
                                    qdec_s[:, 4 * b:4 * b + 4],
                                    start=True, stop=True,
                                    skip_group_check=True)
                        parts.append(qk_slice)

                    def qk_exp():
                        ed = ed_pool.tile([128, SDW], FP16, name="ed")
                        nc.scalar.activation(ed[:, 0:W], state["dec"][:, 0:W],
                                             EXP, scale=SCALE)
                        dec_pending[0] = (g, state["dec"], ed)
                    parts.append(qk_exp)
                    return parts

                def emit_dec_av_parts(g, dec, ed):
                    seqs, coff = grp_seqs[g], grp_coff[g]
                    od = dec[:, SDW:SDW + HEAD_DIM + 1]
                    # col-tiled AV: the 4 seqs' [128tok x 4q] stationaries sit
                    # in distinct 32-col groups of the PE array, so each
                    # round's 4 matmuls stream their V tiles CONCURRENTLY
                    # (~3.3x on this weight-bound stage).
                    # start=True clears has_written for the whole bank, so
                    # only the first matmul of round 0 sets it; the other
                    # seqs' first writes land on cleared bits and overwrite.
                    max_nt = max(ntiles_b[b] for b in seqs)

                    def av_rounds(t0, t1):
                        for t in range(t0, t1):
                            for i in range(NG):
                                b = seqs[i]
                                nt = ntiles_b[b]
                                if t >= nt:
                                    continue
                                gt = stile_off[g * NG + i] + t
                                o = coff[i] + 4 * t
                                nc.tensor.matmul(
                                    od[32 * i:32 * i + GQA, :],
                                    ed[:, o:o + 4],
                                    vp[:, gt, :],
                                    start=(t == 0 and i == 0),
                                    stop=(t == nt - 1),
                                    skip_group_check=True,
                                    tile_position=(0, 32 * i))

                    def av_a():
                        for i in range(NG):
                            b = seqs[i]
                            nt = ntiles_b[b]
                            rem = int(ctx_lens[b]) - 128 * (nt - 1)
                            if rem < 128:
                                o = coff[i] + 4 * (nt - 1)
                                nc.vector.tensor_scalar_mul(
                                    ed[:, o:o + 4], ed[:, o:o + 4],
                                    tail_s[:, b:b + 1])
                        av_rounds(0, (max_nt + 1) // 2)

                    def av_b():
                        av_rounds((max_nt + 1) // 2, max_nt)
                        nc.vector.tensor_copy(dstage[:, g, :], od[:, :])
                        nc.gpsimd.dma_start(ddec[g], dstage[:, g, :])
                    return [av_a, av_b]

                # decode groups ride the back half of the prefill chunk list:
                # a full prefill chunk's matmuls sit between each group's QK
                # and its AV, hiding the exp latency, and the decode work
                # fills the PE slack of the ScalarE-paced prefill chunks.
                n_groups = DECODE_BATCH // NG
                chunk_no = [0]
                dec_pending = [None]

                def decode_slices():
                    k = chunk_no[0]
                    chunk_no[0] += 1
                    # groups ride chunks 3..10: clear of both the
                    # startup-critical loads and the final-chunk tail
                    slices = []
                    if dec_pending[0] is not None:
                        slices += emit_dec_av_parts(*dec_pending[0])
                        dec_pending[0] = None
                    if k in g_of_chunk:
                        slices += emit_dec_qk_parts(g_of_chunk[k])
                    return slices

                for s in range(NUM_SEQS):
                    if s == 1:
                        # must be emitted before chunk 4 (s1,hp0,c0) whose
                        # decode slices read qdec_s/tail_s
                        nc.sync.dma_start(tail_s, tailmask[:, :])
                        nc.sync.dma_start(qdec_s, qdecT[:, :])
                    kT = kT_pool.tile([128, SEQLEN], FP16, name="kT")
                    # one descriptor per tensor: the sync engine issues
                    # descriptors at ~600ns each, so fewer+larger loads get
                    # chunk-0/1's inputs resident sooner (the HAM warmup
                    # bridges the slightly longer first transfer)
                    nc.sync.dma_start(kT, kpreT[s])
                    for hp in range(2):
                        qT = qT_pool.tile([128, 2, SEQLEN], FP16, name="qT")
                        for hh in range(2):
                            nc.sync.dma_start(
                                qT[:, hh, :], qpreT[s, hp, :, hh, :])
                        if hp == 0:
                            v1 = v1_pool.tile(
                                [128, SEQLEN // 128, HEAD_DIM + 1], FP16,
                                name="v1")
                            nc.sync.dma_start(v1, vpre1[s])
                        # the very last chunk runs c=0 (4 k-tiles): shorter
                        # AV+evac+DMA tail after the final exp
                        corder = (1, 0) if (s == NUM_SEQS - 1 and hp == 1) \
                            else (0, 1)
                        for c in corder:
                            emit_chunk(s, hp, c, kT, v1, qT, decode_slices,
                                       last=(s == NUM_SEQS - 1 and hp == 1))
                if dec_pending[0] is not None:
                    for part in emit_dec_av_parts(*dec_pending[0]):
                        part()
                    dec_pending[0] = None
                pop_av(0)

    nc.finalize()
    return nc


def kernel(q, k, v, k_cache, v_cache, slot_mapping, context_lens,
           decode_block_tables, **_unused):
    q = np.asarray(q, dtype=np.float32)
    k = np.asarray(k, dtype=np.float32)
    v = np.asarray(v, dtype=np.float32)
    k_cache = np.asarray(k_cache, dtype=np.float32)
    v_cache = np.asarray(v_cache, dtype=np.float32)
    slot_mapping = np.asarray(slot_mapping)
    context_lens = np.asarray(context_lens)
    decode_block_tables = np.asarray(decode_block_tables)

    # ---- host prep: apply the kv-cache scatter (the reference's
    # _store_kvcache) so decode reads the updated cache ----
    kc = k_cache.reshape(NUM_BLOCKS * BLOCK_SIZE, NUM_KV_HEADS, HEAD_DIM).copy()
    vc = v_cache.reshape(NUM_BLOCKS * BLOCK_SIZE, NUM_KV_HEADS, HEAD_DIM).copy()
    kc[slot_mapping] = k
    vc[slot_mapping] = v
    kc = kc.reshape(NUM_BLOCKS, BLOCK_SIZE, NUM_KV_HEADS, HEAD_DIM)
    vc = vc.reshape(NUM_BLOCKS, BLOCK_SIZE, NUM_KV_HEADS, HEAD_DIM)

    qpre = q[:N_PREFILL].reshape(NUM_SEQS, SEQLEN, NUM_HEADS, HEAD_DIM)
    kpre = k[:N_PREFILL].reshape(NUM_SEQS, SEQLEN, NUM_KV_HEADS, HEAD_DIM)
    vpre = v[:N_PREFILL].reshape(NUM_SEQS, SEQLEN, NUM_KV_HEADS, HEAD_DIM)
    qdec = q[N_PREFILL:]  # [32, 32, 128]

    ones_pre = np.ones((NUM_SEQS, SEQLEN, 1), np.float32)
    trimask = np.broadcast_to(
        (np.arange(128)[:, None, None] <= np.arange(128)[None, None, :]),
        (128, 2, 128)).astype(np.float16)
    ntiles_b = (-(-context_lens.astype(np.int64) // 128)).astype(np.int64)
    rem_b = context_lens.astype(np.int64) - 128 * (ntiles_b - 1)
    tailmask = (np.arange(128)[:, None] < rem_b[None, :]).astype(np.float32)
    nblocks_b = -(-context_lens.astype(np.int64) // BLOCK_SIZE)
    tot_tiles = int(ntiles_b.sum())
    order = _decode_order(context_lens)

    in_maps = []
    for c in range(N_CORES):
        h0 = c * GQA
        # [s, hp, d, hh, q]
        qpreT = np.ascontiguousarray(
            qpre[:, :, h0:h0 + GQA, :]
            .reshape(NUM_SEQS, SEQLEN, 2, 2, HEAD_DIM)
            .transpose(0, 2, 4, 3, 1)).astype(np.float16)
        kpreT = np.ascontiguousarray(
            kpre[:, :, c, :].transpose(0, 2, 1)).astype(np.float16)
        vpre1 = np.ascontiguousarray(
            np.concatenate([vpre[:, :, c, :], ones_pre], axis=2)
            .reshape(NUM_SEQS, SEQLEN // 128, 128, HEAD_DIM + 1)
            .transpose(0, 2, 1, 3)).astype(np.float16)
        qdecT = np.ascontiguousarray(
            qdec[:, h0:h0 + GQA, :].transpose(2, 0, 1)
            .reshape(HEAD_DIM, DECODE_BATCH * GQA)).astype(NP_FP8)
        # decode pages packed at 128-token granularity, in sorted-group
        # order so prefetch slices arrive in consumption order
        kparts, vparts = [], []
        for b in order:
            nb = int(nblocks_b[b])
            ntok = int(ntiles_b[b]) * 128
            kg = kc[decode_block_tables[b, :nb], :, c, :] \
                .reshape(nb * BLOCK_SIZE, HEAD_DIM)[:ntok]
            vg = vc[decode_block_tables[b, :nb], :, c, :] \
                .reshape(nb * BLOCK_SIZE, HEAD_DIM)[:ntok]
            kparts.append(kg)
            vparts.append(
                np.concatenate([vg, np.ones((ntok, 1), np.float32)], axis=1))
        kdec = np.ascontiguousarray(
            np.concatenate(kparts, axis=0).T).astype(NP_FP8)
        vdec = np.ascontiguousarray(
            np.concatenate(vparts, axis=0)
            .reshape(tot_tiles, 128, HEAD_DIM + 1)
            .transpose(1, 0, 2)).astype(NP_FP8)
        in_maps.append({
            "qpreT": qpreT, "kpreT": kpreT, "vpre1": vpre1,
            "qdecT": qdecT, "kdec": kdec, "vdec": vdec, "trimask": trimask,
            "tailmask": tailmask,
        })

    key = (np.ascontiguousarray(context_lens).tobytes()
           + np.ascontiguousarray(decode_block_tables).tobytes())
    nc = _program_cache.get(key)
    if nc is None:
        nc = _build_program(context_lens)
        _program_cache[key] = nc

    res = run_bass_kernel_spmd(nc, in_maps, core_ids=list(range(N_CORES)))

    out = np.empty((TOTAL, NUM_HEADS, HEAD_DIM), np.float32)
    for c in range(N_CORES):
        # prefill: [s, hp, ch, qp, hh*4+ml, 129] -> [s, ch, ml, qp, hp, hh, d]
        po = res.results[c]["preout"].astype(np.float32).reshape(
            NUM_SEQS, 2, 2, 128, 2, 4, HEAD_DIM + 1)
        po = po.transpose(0, 2, 5, 3, 1, 4, 6).reshape(
            N_PREFILL, GQA, HEAD_DIM + 1)
        out[:N_PREFILL, c * GQA:(c + 1) * GQA, :] = \
            po[:, :, :HEAD_DIM] / po[:, :, HEAD_DIM:]
        # decode: [8 groups, 128 rows (32i+gqa head), 129]; group g row
        # 32i+j = seq order[4g+i], head j
        dd = res.results[c]["ddec"].reshape(
            DECODE_BATCH // 4, 4, 32, HEAD_DIM + 1)[:, :, :GQA, :]
        dd = (dd[..., :HEAD_DIM] / dd[..., HEAD_DIM:]).reshape(
            DECODE_BATCH, GQA, HEAD_DIM)
        out[N_PREFILL + np.asarray(order), c * GQA:(c + 1) * GQA, :] = dd
    return out

